# revision 34
# baseline (speedup 1.0000x reference)
"""LoRA MultiheadAttention on 8 Trainium2 NeuronCores (Bass/Tile).

Sharding: core c = (batch n = c//2, head-group hg = c%2); each core handles
6 of 12 heads for one of 4 batches. LoRA is folded into the projection
weights on the host (W_eff = W + scale * up @ down — exact identity).
Inputs ship pre-transposed AND pre-cast to f16 (halves DMA, removes device
casts). Per core: QKV projections in f16; full-softmax attention per head
with the scores for even/odd s-tiles issued to disjoint PE row groups
(partitions 0-63 / 64-127) so they execute concurrently (K=64 row tiling);
attnV is pipelined st-chunk by st-chunk right behind the exps; softmax
denominator rides an extra ones-column in v_aug; normalization uses
reciprocal_approx_fast + gpsimd broadcast. Out-projection partials per
head-pair stream out as f16; the host sums 3 pair-partials x 2 cores per
batch and adds the bias terms (pure unshard glue).
"""
import numpy as np

import concourse.bass as bass
import concourse.tile as tile
from concourse import bacc, mybir
from concourse.bass_utils import run_bass_kernel_spmd

L, N, E, H, R = 2048, 4, 768, 12, 16
ALPHA = 16.0
LORA_SCALE = ALPHA / R
HD = E // H          # 64
HG = 2               # head groups (column-parallel dimension)
HPG = H // HG        # 6 heads per group
EG = E // HG         # 384 columns per group
NC_ = 8
F32 = mybir.dt.float32
F16 = mybir.dt.float16
SCALE = 1.0 / float(np.sqrt(HD))  # folded into exp's input scale
KC = E // 128        # 6 contraction chunks
LT = L // 128        # 16 s-tiles
VW = HPG * (HD + 1)  # 390: per-head 64 v cols + 1 ones col

_CACHED = {}


def _build(debug=False):
    nc = bacc.Bacc()
    xqT = nc.dram_tensor("xqT", [E, L], F16, kind="ExternalInput")
    xkT = nc.dram_tensor("xkT", [E, L], F16, kind="ExternalInput")
    xvT = nc.dram_tensor("xvT", [E, L], F16, kind="ExternalInput")
    wqT = nc.dram_tensor("wqT", [E, EG], F16, kind="ExternalInput")
    wkT = nc.dram_tensor("wkT", [E, EG], F16, kind="ExternalInput")
    wvT = nc.dram_tensor("wvT", [E, EG], F16, kind="ExternalInput")
    woT = nc.dram_tensor("woT", [EG, E], F16, kind="ExternalInput")
    bq = nc.dram_tensor("bq", [EG], F32, kind="ExternalInput")
    bk = nc.dram_tensor("bk", [EG], F32, kind="ExternalInput")
    out = nc.dram_tensor("out", [3, E, L], F16, kind="ExternalOutput")
    if debug:
        dbg_qk = nc.dram_tensor("dbg_qk", [4, 128, L], F16,
                                kind="ExternalOutput")
        dbg_attn = nc.dram_tensor("dbg_attn", [2, 128, L], F16,
                                  kind="ExternalOutput")
        dbg_vaug = nc.dram_tensor("dbg_vaug", [128, VW], F16,
                                  kind="ExternalOutput")
        dbg_x = nc.dram_tensor("dbg_x", [3, 128, L], F16,
                               kind="ExternalOutput")
        dbg_w = nc.dram_tensor("dbg_w", [3, 128, EG], F16,
                               kind="ExternalOutput")

    with tile.TileContext(nc) as tc:
        with (
            tc.tile_pool(name="xp", bufs=18) as xp,
            tc.tile_pool(name="persist", bufs=1) as persist,
            tc.tile_pool(name="attn", bufs=5) as attn_p,
            tc.tile_pool(name="swp", bufs=4) as sw_p,
            tc.tile_pool(name="lnrec", bufs=1) as lnrec,
            tc.tile_pool(name="osb", bufs=3) as osb_p,
            tc.tile_pool(name="psum", bufs=1, space="PSUM") as psum,
        ):
            # ---- weights + biases + x (f16 from host), all on the sync
            # queue, ordered so the lead-in's consumers come first ----
            w16 = {}
            x16 = {}
            bias_t = {}
            wo16 = []

            def load_w(pname, wdram):
                for kk in range(KC):
                    wt = persist.tile([128, EG], F16, name=f"w_{pname}{kk}")
                    nc.sync.dma_start(wt[:], wdram[kk * 128:(kk + 1) * 128, :])
                    w16[pname, kk] = wt

            load_w("q", wqT)
            load_w("k", wkT)
            load_w("v", wvT)
            for bname, bdram in (("q", bq), ("k", bk)):
                for p in range(3):
                    bt = persist.tile([128, 1], F32, name=f"b_{bname}{p}")
                    nc.sync.dma_start(bt[:], bdram[p * 128:(p + 1) * 128])
                    bias_t[bname, p] = bt
            # x loads spread over the 3 DMA-capable queues for parallelism
            for pname, xdram, eng in (("q", xqT, nc.sync),
                                      ("k", xkT, nc.gpsimd),
                                      ("v", xvT, nc.scalar)):
                for kk in range(KC):
                    xt = xp.tile([128, L], F16, tag="x", name=f"x_{pname}{kk}")
                    eng.dma_start(xt[:], xdram[kk * 128:(kk + 1) * 128, :])
                    x16[pname, kk] = xt
            for p in range(3):
                wt = persist.tile([128, E], F16, name=f"wo{p}")
                nc.sync.dma_start(wt[:], woT[p * 128:(p + 1) * 128, :])
                wo16.append(wt)

            v_aug = [None] * LT
            qkT = {}
            qk_swap = {}
            oT = [persist.tile([128, L], F16, name=f"oT{p}") for p in range(3)]

            # ---- background work units (each uses one "sc" psum slot) ----
            def v_unit(st):
                mm = psum.tile([128, 1024], F32, tag="sc", bufs=2, name="mm")
                for kk in range(KC):
                    nc.tensor.matmul(
                        mm[:, 0:EG],
                        x16["v", kk][:, st * 128:(st + 1) * 128],
                        w16["v", kk][:],
                        start=(kk == 0), stop=(kk == KC - 1),
                    )
                vt = persist.tile([128, VW], F16, name=f"v_aug{st}")
                grp = vt.rearrange("p (h c) -> p h c", c=HD + 1)
                nc.vector.tensor_copy(
                    grp[:, :, 0:HD],
                    mm[:, 0:EG].rearrange("p (h c) -> p h c", c=HD),
                )
                nc.vector.memset(grp[:, :, HD:HD + 1], 1.0)
                v_aug[st] = vt

            def qk_unit(pname, p, lc):
                key = (pname, p)
                if key not in qkT:
                    qkT[key] = persist.tile([128, L], F16, name=f"{pname}T{p}")
                dst = qkT[key]
                mm = psum.tile([128, 1024], F32, tag="sc", bufs=2, name="mm")
                for half in range(2):
                    l0 = lc * 1024 + half * 512
                    for kk in range(KC):
                        nc.tensor.matmul(
                            mm[:, half * 512:(half + 1) * 512],
                            w16[pname, kk][:, p * 128:(p + 1) * 128],
                            x16[pname, kk][:, l0:l0 + 512],
                            start=(kk == 0), stop=(kk == KC - 1),
                        )
                nc.vector.tensor_scalar_add(
                    dst[:, lc * 1024:(lc + 1) * 1024], mm[:], bias_t[pname, p][:]
                )

            def swap_unit(p):
                # partition-swapped copies so a single head's q/k exist in
                # BOTH partition halves (for even/odd st row-group pairing)
                for pname in ("q", "k"):
                    s = sw_p.tile([128, L], F16, tag="sw", name=f"{pname}sw{p}")
                    nc.vector.tensor_copy(s[0:64, :], qkT[pname, p][64:128, :])
                    nc.vector.tensor_copy(s[64:128, :], qkT[pname, p][0:64, :])
                    qk_swap[pname, p] = s

            def out_unit(p, j, lc):
                mm = psum.tile([128, 1024], F32, tag="sc", bufs=2, name="mm")
                for half in range(2):
                    eo = 2 * j + half
                    nc.tensor.matmul(
                        mm[:, half * 512:(half + 1) * 512],
                        wo16[p][:, eo * 128:(eo + 1) * 128],
                        oT[p][:, lc * 512:(lc + 1) * 512],
                        start=True, stop=True,
                    )
                osb = osb_p.tile([128, 1024], F16, tag="osb", name="osb")
                nc.vector.tensor_copy(osb[:], mm[:])
                for half in range(2):
                    eo = 2 * j + half
                    nc.sync.dma_start(
                        out[p, eo * 128:(eo + 1) * 128,
                            lc * 512:(lc + 1) * 512],
                        osb[:, half * 512:(half + 1) * 512])

            # ---- schedule ----
            # lead-in: pair-0 q/k proj + swaps + v st 0..3
            for lc in range(2):
                qk_unit("q", 0, lc)
            for lc in range(2):
                qk_unit("k", 0, lc)
            swap_unit(0)
            for st in range(4):
                v_unit(st)

            # background units per global iteration (48 = 6 heads x 8)
            slots = {}
            for i in range(4):                       # v4..v11 doubled
                slots[i] = [lambda st=4 + 2 * i: v_unit(st),
                            lambda st=5 + 2 * i: v_unit(st)]
            for i in range(4):                       # v12..v15
                slots[4 + i] = [lambda st=12 + i: v_unit(st)]
            slots[8] = [lambda: qk_unit("q", 1, 0)]
            slots[9] = [lambda: qk_unit("q", 1, 1)]
            slots[10] = [lambda: qk_unit("k", 1, 0)]
            slots[11] = [lambda: qk_unit("k", 1, 1)]
            slots[12] = [lambda: swap_unit(1)]
            # out0 (12 units): after head-1 epilogue -> iters >= 16
            for k in range(12):
                j, lc = divmod(k, 4)
                slots[16 + k] = [lambda j=j, lc=lc: out_unit(0, j, lc)]
            slots[28] = [lambda: qk_unit("q", 2, 0),
                         lambda: qk_unit("q", 2, 1)]
            slots[29] = [lambda: qk_unit("k", 2, 0),
                         lambda: qk_unit("k", 2, 1)]
            slots[30] = [lambda: swap_unit(2)]
            # out1 (12 units): after head-3 epilogue -> iters >= 32
            for k in range(12):
                j, lc = divmod(k, 4)
                slots[32 + k] = [lambda j=j, lc=lc: out_unit(1, j, lc)]

            giter = 0
            for h in range(HPG):
                p, hw = h // 2, h % 2
                pb = hw * 64
                qo, ko = qkT["q", p], qkT["k", p]
                qs, ks = qk_swap["q", p], qk_swap["k", p]
                if hw == 0:
                    lo = (qo, ko, 0)     # rows 0-63 (row group A)
                    hi = (qs, ks, 64)    # rows 64-127 (row group B)
                else:
                    lo = (qs, ks, 0)
                    hi = (qo, ko, 64)
                ot = [psum.tile([65, 512], F32, tag="ot", bufs=4,
                                name=f"ot{lc}") for lc in range(4)]
                pending = None
                for stp in range(8):
                    st0, st1 = 2 * stp, 2 * stp + 1
                    at_e = attn_p.tile([128, L], F16, tag="attn", name="at_e")
                    at_o = attn_p.tile([128, L], F16, tag="attn", name="at_o")
                    for lc in range(2):
                        sc_e = psum.tile([128, 1024], F32, tag="sc", bufs=2,
                                         name="sc_e")
                        sc_o = psum.tile([128, 1024], F32, tag="sc", bufs=2,
                                         name="sc_o")
                        for half in range(2):
                            l0 = lc * 1024 + half * 512
                            sl = slice(half * 512, (half + 1) * 512)
                            qt, kt, rb = lo
                            nc.tensor.matmul(
                                sc_e[:, sl],
                                kt[rb:rb + 64, st0 * 128:(st0 + 1) * 128],
                                qt[rb:rb + 64, l0:l0 + 512],
                                start=True, stop=True,
                            )
                            qt, kt, rb = hi
                            nc.tensor.matmul(
                                sc_o[:, sl],
                                kt[rb:rb + 64, st1 * 128:(st1 + 1) * 128],
                                qt[rb:rb + 64, l0:l0 + 512],
                                start=True, stop=True,
                            )
                        nc.scalar.activation(
                            at_e[:, lc * 1024:(lc + 1) * 1024], sc_e[:],
                            mybir.ActivationFunctionType.Exp, scale=SCALE)
                        nc.scalar.activation(
                            at_o[:, lc * 1024:(lc + 1) * 1024], sc_o[:],
                            mybir.ActivationFunctionType.Exp, scale=SCALE)
                    # software pipeline: attnV runs one iteration behind
                    if pending is not None:
                        pending()
                    for fn in slots.get(giter, ()):
                        fn()

                    def attnv(h=h, st0=st0, st1=st1, at_e=at_e, at_o=at_o):
                        for st, at in ((st0, at_e), (st1, at_o)):
                            for lc4 in range(4):
                                nc.tensor.matmul(
                                    ot[lc4][:],
                                    v_aug[st][:, h * (HD + 1):
                                              (h + 1) * (HD + 1)],
                                    at[:, lc4 * 512:(lc4 + 1) * 512],
                                    start=(st == 0), stop=(st == LT - 1),
                                )
                    pending = attnv
                    if debug and h == 0 and stp == 0:
                        for di, pn in enumerate(("q", "k", "v")):
                            nc.sync.dma_start(dbg_x[di], x16[pn, 0][:])
                            nc.sync.dma_start(dbg_w[di], w16[pn, 0][:])
                        nc.sync.dma_start(dbg_attn[0], at_e[:])
                        nc.sync.dma_start(dbg_attn[1], at_o[:])
                        nc.sync.dma_start(dbg_qk[0], qkT["q", 0][:])
                        nc.sync.dma_start(dbg_qk[1], qkT["k", 0][:])
                        nc.sync.dma_start(dbg_qk[2], qk_swap["q", 0][:])
                        nc.sync.dma_start(dbg_qk[3], qk_swap["k", 0][:])
                        nc.sync.dma_start(dbg_vaug[:], v_aug[0][:])
                    giter += 1
                pending()
                # epilogue: 1/den = exp(-ln(den)) on ACT (same act table as
                # the softmax exp), broadcast, normalize into oT
                ln32 = lnrec.tile([1, L], F32, tag="ln", bufs=1, name="ln32")
                for lc4 in range(4):
                    nc.scalar.activation(
                        ln32[0:1, lc4 * 512:(lc4 + 1) * 512],
                        ot[lc4][64:65, :],
                        mybir.ActivationFunctionType.Ln)
                rec32 = lnrec.tile([1, L], F32, tag="rec", bufs=1,
                                   name="rec32")
                nc.scalar.activation(
                    rec32[:], ln32[:],
                    mybir.ActivationFunctionType.Exp, scale=-1.0)
                rbc32 = lnrec.tile([64, L], F32, tag="rbc", bufs=1,
                                   name="rbc32")
                nc.gpsimd.partition_broadcast(rbc32[:], rec32[:])
                for lc4 in range(4):
                    nc.vector.tensor_mul(
                        oT[p][pb:pb + 64, lc4 * 512:(lc4 + 1) * 512],
                        ot[lc4][0:64, :],
                        rbc32[:, lc4 * 512:(lc4 + 1) * 512])

            # tail: pair-2 out-projection
            for j in range(3):
                for lc in range(4):
                    out_unit(2, j, lc)
    nc.finalize()
    return nc


def kernel(query, key, value, in_proj_weight, in_proj_bias,
           q_down, q_up, k_down, k_up, v_down, v_up,
           out_proj_weight, out_proj_bias, out_down, out_up):
    if "nc" not in _CACHED:
        _CACHED["nc"] = _build()
    nc = _CACHED["nc"]

    f, f16 = np.float32, np.float16
    # fold LoRA into the projection weights (exact algebraic identity)
    w_eff = {}
    for i, (dn, up) in enumerate(((q_down, q_up), (k_down, k_up),
                                  (v_down, v_up))):
        w = in_proj_weight[i * E:(i + 1) * E].astype(f)
        w_eff[i] = w + LORA_SCALE * (up.astype(f) @ dn.astype(f))
    wo_eff = out_proj_weight.astype(f) + LORA_SCALE * (
        out_up.astype(f) @ out_down.astype(f))

    in_maps = []
    for c in range(NC_):
        n, hg = c // 2, c % 2
        sl = slice(hg * EG, (hg + 1) * EG)
        m = {
            "xqT": np.ascontiguousarray(query[:, n, :].T).astype(f16),
            "xkT": np.ascontiguousarray(key[:, n, :].T).astype(f16),
            "xvT": np.ascontiguousarray(value[:, n, :].T).astype(f16),
            "wqT": np.ascontiguousarray(w_eff[0][sl].T).astype(f16),
            "wkT": np.ascontiguousarray(w_eff[1][sl].T).astype(f16),
            "wvT": np.ascontiguousarray(w_eff[2][sl].T).astype(f16),
            "woT": np.ascontiguousarray(wo_eff[:, sl].T).astype(f16),
            "bq": np.ascontiguousarray(in_proj_bias[0:E][sl], dtype=f),
            "bk": np.ascontiguousarray(in_proj_bias[E:2 * E][sl], dtype=f),
        }
        in_maps.append(m)

    _CACHED["in_maps"] = in_maps
    res = run_bass_kernel_spmd(nc, in_maps, list(range(NC_)))
    outp = np.empty((L, N, E), dtype=np.float32)
    bo_total = out_proj_bias.astype(f) + wo_eff @ np.ascontiguousarray(
        in_proj_bias[2 * E:3 * E], dtype=f)
    for n in range(N):
        acc = (res.results[2 * n]["out"].astype(f).sum(axis=0)
               + res.results[2 * n + 1]["out"].astype(f).sum(axis=0))
        outp[:, n, :] = acc.T + bo_total
    return outp
